# revision 15
# baseline (speedup 1.0000x reference)
"""Trainium2 Bass kernel for the HardResetSSMBlock problem.

y = silu(x @ W1 + b1) @ W2 + b2, masked per frame, with a periodic decay
scale on frames where (t+1) % 10 == 0.

Strategy: the mask zeroes ~half the output tokens, and the op is
stateless per token, so the host packs only the unmasked tokens into a
dense stream (pure data movement -- all FLOPs stay on device), splits
it evenly across 8 NeuronCores, and scatters the device results back
into a zero-filled output. This halves both HBM traffic and compute.
All device HBM traffic is bf16 (PSUM accumulation stays f32; ~4e-3
max rel err). The remaining per-token scale fed to the device is the
decay factor at each kept token's original position.

Device pipeline per 1024-token group (2048-token tiles = 2 groups):
  MM1 (W1 stationary, X moving, 2x N=512) into a 2-bank PSUM group ->
  Silu(+b1) on ACT writing bf16 -> MM2 with 128-token H^T chunks
  stationary (FWL, bf16) so Y lands token-major, two 512-token halves
  into 1-bank PSUM buffers -> decay scale via per-partition broadcast
  tensor_mul on DVE, f32 PSUM -> bf16 SBUF -> 512KB DMA out per tile.
MM2 is skewed two groups behind MM1 so the ACT latency is hidden from
the PE; ~88 junk matmuls + a dummy activation during the DMA fill warm
the PE clock (HAM) and preload the silu spline tables.

The device kernel is compiled for ceil(max_core_tokens/2048) tiles and
cached per tile count, so any mask density (including all-ones) works.
"""

import numpy as np

B, S, D = 16, 16384, 128
RESET_PERIOD = 10
DECAY_FACTOR = 0.1
N_CORES = 8
TILE_TOK = 4096
CH = TILE_TOK // 128  # 16 chunks of 128 tokens
GRP = 1024  # tokens per PSUM group (2 banks)
G_PER_TILE = TILE_TOK // GRP  # 2
PREFETCH = 2
SKEW = 2  # groups of MM1->ACT lead before MM2 drains

# Silu on hardware; CoreSim lacks it, so sim tests may override (e.g. Sigmoid)
ACT_FUNC = "Silu"

_CACHE = {}


def _build_nc(n_tiles):
    import concourse.bacc as bacc
    import concourse.tile as tile
    from concourse import mybir
    from concourse.bass import AP

    f32 = mybir.dt.float32
    bf16 = mybir.dt.bfloat16
    n_grp = n_tiles * G_PER_TILE
    n_chunks = n_tiles * CH
    act_fn = getattr(mybir.ActivationFunctionType, ACT_FUNC)

    nc = bacc.Bacc()
    xt_d = nc.dram_tensor(
        "x_t", [n_tiles, 128, TILE_TOK], bf16, kind="ExternalInput"
    )
    st_d = nc.dram_tensor("s_t", [128, n_chunks], f32, kind="ExternalInput")
    w1_d = nc.dram_tensor("w1", [D, D], bf16, kind="ExternalInput")
    w2_d = nc.dram_tensor("w2", [D, D], bf16, kind="ExternalInput")
    b1_d = nc.dram_tensor("b1", [D, 1], f32, kind="ExternalInput")
    # partition-major output tiles: y_t[t, p, c, d] = y[(t*CH + c)*128 + p, d]
    # (host un-permutes; this makes the out-DMA write 4KB contiguous runs)
    y_d = nc.dram_tensor(
        "y_t", [n_tiles, 128, CH, D], bf16, kind="ExternalOutput"
    )

    with tile.TileContext(nc) as tc:
        with (
            tc.tile_pool(name="const", bufs=1) as constp,
            tc.tile_pool(name="data", bufs=1) as datap,
            tc.tile_pool(name="ps_ht", bufs=SKEW + 1, space="PSUM") as ps_htp,
            tc.tile_pool(name="ps_y", bufs=2, space="PSUM") as ps_yp,
        ):
            # --- warmup: keep the PE busy during the DMA fill so HAM
            # unthrottles (2.4 GHz) before the first real matmul, and
            # preload the silu spline tables on ACT.
            junk = constp.tile([128, 128], bf16, name="junk")
            nc.vector.memset(junk[:], 0.0)
            p_j = ps_yp.tile([128, GRP // 256, 128], f32, name="p_j")
            nc.tensor.matmul(
                p_j[:, 0, :], junk[:], junk[:], start=True, stop=True
            )
            p_j2 = ps_yp.tile([128, GRP // 256, 128], f32, name="p_j")
            for _ in range(29):
                nc.tensor.matmul(
                    p_j2[:, 0, :], junk[:], junk[:], start=True, stop=True
                )

            b1_s = constp.tile([128, 1], f32)
            nc.sync.dma_start(b1_s[:], b1_d[:])
            w1_s = constp.tile([128, 128], bf16)
            nc.sync.dma_start(w1_s[:], w1_d[:])
            w2_s = constp.tile([128, 128], bf16)
            nc.gpsimd.dma_start(w2_s[:], w2_d[:])
            st_s = constp.tile([128, n_chunks], f32)

            x_tiles = [None] * n_tiles
            y_tiles = [None] * n_tiles
            h_grp = [None] * n_grp

            def in_dma(t, split=False):
                if t >= n_tiles:
                    return
                x_tiles[t] = datap.tile(
                    [128, TILE_TOK], bf16, name="s_xt", bufs=PREFETCH + 1
                )
                if split:
                    nc.sync.dma_start(
                        x_tiles[t][:, :GRP], xt_d[t, :, :GRP]
                    )
                    nc.scalar.dma_start(
                        x_tiles[t][:, GRP:], xt_d[t, :, GRP:]
                    )
                    return
                else:
                    eng = nc.sync if t % 2 == 0 else nc.scalar
                    eng.dma_start(x_tiles[t][:], xt_d[t])

            in_dma(0, split=True)
            nc.gpsimd.dma_start(st_s[:], st_d[:])
            for t in range(1, PREFETCH):
                in_dma(t)

            # preload the silu spline tables: same (PSUM-source, bf16-out,
            # with-bias) activation variant as the real ones so the
            # ACT_TABLE_LOAD attaches here, during the DMA fill
            actwarm = constp.tile([128, 1], bf16, name="actwarm")
            nc.scalar.activation(
                actwarm[:], p_j[:, 0, 0:1], act_fn, bias=b1_s[:], scale=1.0
            )

            # software-pipelined: PE order is MM1(0), MM1(1), MM1(2),
            # MM2(0), MM1(3), MM2(1), ... so ACT(g) latency is hidden.
            for g in range(n_grp + SKEW):
                if g < n_grp:
                    t = g // G_PER_TILE
                    if g % G_PER_TILE == 0:
                        in_dma(t + PREFETCH)
                        y_tiles[t] = datap.tile(
                            [128, CH, D], bf16, name="s_y", bufs=2
                        )
                    s_xt = x_tiles[t]
                    off = (g % G_PER_TILE) * GRP
                    ps = ps_htp.tile([128, GRP], f32)
                    for h in range(GRP // 512):
                        hs = slice(off + h * 512, off + (h + 1) * 512)
                        nc.tensor.matmul(
                            ps[:, h * 512:(h + 1) * 512], w1_s[:],
                            s_xt[:, hs], start=True, stop=True,
                        )
                    h_grp[g] = datap.tile([128, GRP], bf16, name="s_h", bufs=3)
                    nc.scalar.activation(
                        h_grp[g][:], ps[:], act_fn, bias=b1_s[:], scale=1.0
                    )

                if g >= SKEW:
                    gp = g - SKEW
                    tp = gp // G_PER_TILE
                    for half in range(2):
                        c0 = (gp % G_PER_TILE) * (GRP // 128) + half * 4
                        p_y = ps_yp.tile([128, 4, 128], f32, name="p_j")
                        for c in range(4):
                            cc = half * 4 + c
                            nc.tensor.matmul(
                                p_y[:, c, :],
                                h_grp[gp][:, cc * 128:(cc + 1) * 128], w2_s,
                                start=True, stop=True,
                            )
                        s_slice = st_s[:, tp * CH + c0:tp * CH + c0 + 4]
                        s_bcast = AP(
                            tensor=s_slice.tensor,
                            offset=s_slice.offset,
                            ap=list(s_slice.ap) + [[0, 128]],
                        )  # [128, 4, 128] with stride-0 feature dim
                        nc.vector.tensor_mul(
                            y_tiles[tp][:, c0:c0 + 4, :], p_y[:], s_bcast
                        )
                    if gp % G_PER_TILE == G_PER_TILE - 1:
                        if tp == n_tiles - 1:
                            # split the last write across two queues to
                            # shorten the final drain
                            half_ch = CH // 2
                            nc.scalar.dma_start(
                                y_d[tp, :, :half_ch], y_tiles[tp][:, :half_ch]
                            )
                            nc.sync.dma_start(
                                y_d[tp, :, half_ch:], y_tiles[tp][:, half_ch:]
                            )
                        else:
                            nc.sync.dma_start(y_d[tp], y_tiles[tp][:])

    nc.finalize()
    return nc


def _get_nc(n_tiles):
    key = ("nc", n_tiles)
    if key not in _CACHE:
        _CACHE[key] = _build_nc(n_tiles)
    return _CACHE[key]


def kernel(x, mask, W1, b1, W2, b2, _trace=False):
    from ml_dtypes import bfloat16
    from concourse.bass_utils import run_bass_kernel_spmd

    x = np.asarray(x, dtype=np.float32)
    mask = np.asarray(mask)
    W1b = np.ascontiguousarray(np.asarray(W1, dtype=np.float32)).astype(
        bfloat16
    )
    W2b = np.ascontiguousarray(np.asarray(W2, dtype=np.float32)).astype(
        bfloat16
    )
    b1v = np.asarray(b1, dtype=np.float32).reshape(D, 1)
    b2 = np.asarray(b2, dtype=np.float32)

    t = np.arange(S)
    decay = np.where((t + 1) % RESET_PERIOD == 0, DECAY_FACTOR, 1.0).astype(
        np.float32
    )

    # pack unmasked tokens into a dense stream, split evenly over cores
    mask_flat = mask.reshape(-1)
    idx = np.flatnonzero(mask_flat)
    K = idx.size
    out_flat = np.zeros((B * S, D), dtype=np.float32)
    if K:
        k8 = -(-K // N_CORES)
        n_tiles = max(1, -(-k8 // TILE_TOK))
        cap = n_tiles * TILE_TOK
        tot = cap * N_CORES

        xp = np.zeros((tot, D), dtype=bfloat16)
        xp[:K] = x.reshape(B * S, D)[idx]
        sp = np.zeros(tot, dtype=np.float32)
        sp[:K] = np.broadcast_to(decay[None, :], (B, S)).reshape(-1)[idx]

        # feature-major tiles: [core, n_tiles, 128(d), TILE_TOK]
        x_t_all = np.ascontiguousarray(
            xp.reshape(N_CORES, n_tiles, TILE_TOK, D).transpose(0, 1, 3, 2)
        )
        s_all = sp.reshape(N_CORES, cap // 128, 128)

        in_maps = []
        for c in range(N_CORES):
            s_t = np.ascontiguousarray(s_all[c].T)  # [128, n_chunks]
            in_maps.append(
                {
                    "x_t": x_t_all[c],
                    "s_t": s_t,
                    "w1": W1b,
                    "w2": W2b,
                    "b1": b1v,
                }
            )

        nc = _get_nc(n_tiles)
        res = run_bass_kernel_spmd(
            nc, in_maps, list(range(N_CORES)), trace=_trace
        )
        if _trace:
            _CACHE["last_results"] = res
        # y_t[t, p, c, d] -> packed token (t*CH + c)*128 + p
        yp = np.stack(
            [np.asarray(res.results[c]["y_t"]) for c in range(N_CORES)]
        )
        yp = (
            yp.transpose(0, 1, 3, 2, 4)
            .astype(np.float32)
            .reshape(N_CORES * cap, D)
        )
        out_flat[idx] = yp[:K]

    out = out_flat.reshape(B, S, D)
    if np.any(b2):
        # device computes (h @ W2) * s; the masked/decayed bias is added here
        s = mask.astype(np.float32) * decay[None, :]
        out = out + s[:, :, None] * b2[None, None, :]
    return out
